# revision 4
# baseline (speedup 1.0000x reference)
"""ClassicalSelfAttention TRN2 kernel — 8-core SPMD, sequence-parallel.

out = softmax((X Wq)(X Wk)^T / sqrt(d)) @ X,  X:[4096,1024] f32, W:[1024,1024].

Strategy (per core, rows sharded 8x512):
  A   = Wq @ Wk^T                (replicated, fp16x2 split matmuls)
  B^T = A^T-contraction form:  B^T[e,m] = sum_d A[d,e] Xl^T[d,m]   (fp16x2)
  S   = B @ X^T  via lhsT=B^T tiles, rhs=X^T chunks                (fp16x2)
  P   = softmax(S/32) row-wise (2-pass, ACT exp with per-partition bias)
  out = (P @ X) * (1/rowsum)   (fp32r matmuls, PSUM fp32 accumulate)

All transposes on PE (fp32, via identity), hi/lo fp16 split happens on the
PSUM->SBUF copy-out (DVE). Logit precision ~ fp32-grade (bf16 single-pass
flips argmax rows here; see softmax sharpness: top-2 logit gaps down to 3e-3).
P^T is spilled to DRAM and streamed back during PV to keep SBUF under budget.
"""
import numpy as np
import concourse.bass as bass
import concourse.bacc as bacc
import concourse.mybir as mybir
import concourse.tile as tile
from concourse import masks
from concourse.bass_utils import run_bass_kernel_spmd

F32 = mybir.dt.float32
F32R = mybir.dt.float32r
F16 = mybir.dt.float16

D = 1024          # embed dim
NT = 4096         # tokens
NC = 8            # cores
NL = NT // NC     # 512 local rows
DT = D // 128     # 8 d-tiles
JC = NT // 512    # 8 j-chunks
MT = NL // 128    # 4 m-tiles
SCALE = float(1.0 / np.sqrt(np.float32(D)))

EXP = mybir.ActivationFunctionType.Exp
COPY = mybir.ActivationFunctionType.Copy


def _split_copy(nc, psrc, hdst, ldst):
    """psum f32 -> hdst f16 (round) and ldst f16 (residual), both on DVE."""
    nc.vector.tensor_copy(hdst, psrc)
    nc.vector.tensor_sub(ldst, psrc, hdst)


def build_nc():
    nc = bacc.Bacc("TRN2", target_bir_lowering=False, debug=False)

    x_full = nc.declare_dram_parameter("x_full", [NT, D], F32, isOutput=False)
    x_local = nc.declare_dram_parameter("x_local", [NL, D], F32, isOutput=False)
    wq = nc.declare_dram_parameter("wq", [D, D], F32, isOutput=False)
    wk = nc.declare_dram_parameter("wk", [D, D], F32, isOutput=False)
    out_l = nc.declare_dram_parameter("out_local", [NL, D], F32, isOutput=True)
    pt_dram = nc.dram_tensor("pt_scratch", [NT, NL], F32R)

    with tile.TileContext(nc) as tc:
        with (
            tc.tile_pool(name="persist", bufs=1) as persist,
            tc.tile_pool(name="stream", bufs=4) as stream,
            tc.tile_pool(name="stats", bufs=1) as stats,
        ):
            ident = persist.tile([128, 128], F32, tag="ident", name="ident")
            masks.make_identity(nc, ident[:])

            # ---- stats tiles ----
            pmax = [stats.tile([128, JC], F32, tag=f"pmax{m}", name=f"pmax{m}") for m in range(MT)]
            esum = [stats.tile([128, JC], F32, tag=f"esum{m}", name=f"esum{m}") for m in range(MT)]
            recip = stats.tile([128, MT], F32, tag="recip", name="recip")

            with (
                tc.tile_pool(name="psum1", bufs=2, space=bass.MemorySpace.PSUM) as ptp,
                tc.tile_pool(name="psum2", bufs=4, space=bass.MemorySpace.PSUM) as pacc,
            ):
                # ---------- persistent mid-life tensors ----------
                with tc.tile_pool(name="abuf", bufs=1) as abuf:
                    A_h = [abuf.tile([128, D], F16, tag=f"Ah{r}", name=f"Ah{r}") for r in range(DT)]
                    A_l = [abuf.tile([128, D], F16, tag=f"Al{r}", name=f"Al{r}") for r in range(DT)]

                    # ---------- P1: W^T hi/lo ----------
                    with tc.tile_pool(name="wt", bufs=1) as wtp:
                        wT = {}
                        for wname in ("q", "k"):
                            for h in ("h", "l"):
                                wT[wname + h] = [
                                    wtp.tile([128, DT, 128], F16, tag=f"w{wname}{h}{c}", name=f"w{wname}{h}{c}")
                                    for c in range(DT)
                                ]
                        for wname, wdram in (("q", wq), ("k", wk)):
                            for r in range(DT):
                                wrow = stream.tile([128, D], F32, tag="row", name="wrow")
                                nc.sync.dma_start(wrow[:], wdram[r * 128:(r + 1) * 128, :])
                                for c in range(DT):
                                    pt = ptp.tile([128, 128], F32, tag="tp", name="tp")
                                    nc.tensor.transpose(pt[:], wrow[:, c * 128:(c + 1) * 128], ident[:])
                                    _split_copy(nc, pt[:], wT[wname + "h"][c][:, r, :], wT[wname + "l"][c][:, r, :])

                        # ---------- P2: A = Wq @ Wk^T  (fp16x2) ----------
                        for r in range(DT):
                            for ec in range(2):
                                pa = pacc.tile([128, 512], F32, tag="acc", name="acc")
                                n_mm = 0
                                for c in range(DT):
                                    for lh, rh in (("h", "h"), ("h", "l"), ("l", "h")):
                                        nc.tensor.matmul(
                                            pa[:],
                                            wT["q" + lh][c][:, r, :],
                                            wT["k" + rh][c][:, ec * 4:(ec + 1) * 4, :],
                                            start=(n_mm == 0), stop=(n_mm == 23),
                                        )
                                        n_mm += 1
                                _split_copy(nc, pa[:], A_h[r][:, ec * 512:(ec + 1) * 512],
                                            A_l[r][:, ec * 512:(ec + 1) * 512])

                    # ---------- P3: x_local^T hi/lo ----------
                    with tc.tile_pool(name="btbuf", bufs=1) as btbuf:
                        xlT_h = [btbuf.tile([128, MT, 128], F16, tag=f"xlTh{d}", name=f"xlTh{d}") for d in range(DT)]
                        xlT_l = [btbuf.tile([128, MT, 128], F16, tag=f"xlTl{d}", name=f"xlTl{d}") for d in range(DT)]
                        BT_h = [btbuf.tile([128, MT, 128], F16, tag=f"BTh{e}", name=f"BTh{e}") for e in range(DT)]
                        BT_l = [btbuf.tile([128, MT, 128], F16, tag=f"BTl{e}", name=f"BTl{e}") for e in range(DT)]

                        for js in range(MT):
                            xr = stream.tile([128, D], F32, tag="row", name="xrow")
                            nc.sync.dma_start(xr[:], x_local[js * 128:(js + 1) * 128, :])
                            for d in range(DT):
                                pt = ptp.tile([128, 128], F32, tag="tp", name="tp")
                                nc.tensor.transpose(pt[:], xr[:, d * 128:(d + 1) * 128], ident[:])
                                _split_copy(nc, pt[:], xlT_h[d][:, js, :], xlT_l[d][:, js, :])

                        # ---------- P4: B^T[e,m] = sum_d A[d,e] xlT[d,m] ----------
                        for e in range(DT):
                            pb = pacc.tile([128, 512], F32, tag="acc", name="acc")
                            n_mm = 0
                            for d in range(DT):
                                for lh, rh in (("h", "h"), ("h", "l"), ("l", "h")):
                                    lhsT = (A_h if lh == "h" else A_l)[d][:, e * 128:(e + 1) * 128]
                                    rhs = (xlT_h if rh == "h" else xlT_l)[d][:]
                                    nc.tensor.matmul(pb[:], lhsT, rhs,
                                                     start=(n_mm == 0), stop=(n_mm == 23))
                                    n_mm += 1
                            _split_copy(nc, pb[:], BT_h[e][:], BT_l[e][:])

                        # ---------- P5: S chunks + running max ----------
                        with (
                            tc.tile_pool(name="xt", bufs=2) as xtp,
                            tc.tile_pool(name="sbig", bufs=1) as sbig,
                            tc.tile_pool(name="pst", bufs=2) as pstp,
                        ):
                            S = [sbig.tile([128, JC, 512], F32, tag=f"S{m}", name=f"S{m}") for m in range(MT)]
                            for jc in range(JC):
                                xT_h = xtp.tile([128, DT, 512], F16, tag="xTh", name="xTh")
                                xT_l = xtp.tile([128, DT, 512], F16, tag="xTl", name="xTl")
                                for js in range(4):
                                    xr = stream.tile([128, D], F32, tag="row", name="xrow")
                                    nc.sync.dma_start(xr[:], x_full[jc * 512 + js * 128:jc * 512 + (js + 1) * 128, :])
                                    for d in range(DT):
                                        pt = ptp.tile([128, 128], F32, tag="tp", name="tp")
                                        nc.tensor.transpose(pt[:], xr[:, d * 128:(d + 1) * 128], ident[:])
                                        _split_copy(nc, pt[:], xT_h[:, d, js * 128:(js + 1) * 128],
                                                    xT_l[:, d, js * 128:(js + 1) * 128])
                                for m in range(MT):
                                    ps = pacc.tile([128, 512], F32, tag="acc", name="acc")
                                    n_mm = 0
                                    for e in range(DT):
                                        for lh, rh in (("h", "h"), ("h", "l"), ("l", "h")):
                                            lhsT = (BT_h if lh == "h" else BT_l)[e][:, m, :]
                                            rhs = (xT_h if rh == "h" else xT_l)[:, e, :]
                                            nc.tensor.matmul(ps[:], lhsT, rhs,
                                                             start=(n_mm == 0), stop=(n_mm == 23))
                                            n_mm += 1
                                    nc.scalar.activation(S[m][:, jc, :], ps[:], COPY)
                                    nc.vector.reduce_max(pmax[m][:, jc:jc + 1], ps[:],
                                                         axis=mybir.AxisListType.X)

                            # ---------- P6: softmax + P^T (spill to DRAM) ----------
                            for m in range(MT):
                                rowmax = stats.tile([128, 1], F32, tag=f"rmax{m}", name=f"rmax{m}")
                                nc.vector.reduce_max(rowmax[:], pmax[m][:],
                                                     axis=mybir.AxisListType.X)
                                negb = stats.tile([128, 1], F32, tag=f"negb{m}", name=f"negb{m}")
                                nc.vector.tensor_scalar_mul(negb[:], rowmax[:], -SCALE)
                                for jc in range(JC):
                                    pchunk = pstp.tile([128, 512], F32, tag="pchunk", name="pchunk")
                                    nc.scalar.activation(pchunk[:], S[m][:, jc, :], EXP,
                                                         bias=negb[:], scale=SCALE,
                                                         accum_out=esum[m][:, jc:jc + 1])
                                    ptst = pstp.tile([128, 4, 128], F32R, tag="ptst", name="ptst")
                                    for js in range(4):
                                        pt = ptp.tile([128, 128], F32, tag="tp", name="tp")
                                        nc.tensor.transpose(pt[:], pchunk[:, js * 128:(js + 1) * 128], ident[:])
                                        nc.vector.tensor_copy(ptst[:, js, :], pt[:])
                                    nc.sync.dma_start(
                                        pt_dram[jc * 512:(jc + 1) * 512, m * 128:(m + 1) * 128]
                                        .rearrange("(js p) m -> p js m", p=128),
                                        ptst[:],
                                    )
                                rs = stats.tile([128, 1], F32, tag=f"rs{m}", name=f"rs{m}")
                                nc.vector.reduce_sum(rs[:], esum[m][:], axis=mybir.AxisListType.X)
                                nc.vector.reciprocal(recip[:, m:m + 1], rs[:])

            # ---------- P7: out = (P @ V) * recip   (fp32r) ----------
            with tc.tile_pool(name="pv", bufs=1, space=bass.MemorySpace.PSUM) as pvp:
                with tc.tile_pool(name="ptin", bufs=4) as ptin, tc.tile_pool(name="p7s", bufs=4) as p7s:
                    ppv = [[pvp.tile([128, 512], F32, tag=f"pv{m}_{n}", name=f"pv{m}_{n}") for n in range(2)]
                           for m in range(MT)]
                    for jt in range(NT // 128):
                        vt = p7s.tile([128, D], F32R, tag="vt", name="vt")
                        nc.sync.dma_start(vt[:], x_full[jt * 128:(jt + 1) * 128, :].bitcast(F32R))
                        ptt = ptin.tile([128, NL], F32R, tag="ptt", name="ptt")
                        nc.sync.dma_start(ptt[:], pt_dram[jt * 128:(jt + 1) * 128, :])
                        for m in range(MT):
                            for n in range(2):
                                nc.tensor.matmul(
                                    ppv[m][n][:],
                                    ptt[:, m * 128:(m + 1) * 128],
                                    vt[:, n * 512:(n + 1) * 512],
                                    start=(jt == 0), stop=(jt == NT // 128 - 1),
                                )
                    for m in range(MT):
                        for n in range(2):
                            osb = p7s.tile([128, 512], F32, tag="osb", name="osb")
                            nc.vector.tensor_scalar_mul(osb[:], ppv[m][n][:], recip[:, m:m + 1])
                            nc.sync.dma_start(
                                out_l[m * 128:(m + 1) * 128, n * 512:(n + 1) * 512], osb[:])

    nc.compile()
    return nc


_NC_CACHE = None


def kernel(inputs, rotation_params, entangle_params):
    global _NC_CACHE
    if _NC_CACHE is None:
        _NC_CACHE = build_nc()
    nc = _NC_CACHE
    x = np.ascontiguousarray(np.asarray(inputs, np.float32))
    wq = np.ascontiguousarray(np.asarray(rotation_params, np.float32))
    wk = np.ascontiguousarray(np.asarray(entangle_params, np.float32))
    in_maps = [
        {"x_full": x, "x_local": x[c * NL:(c + 1) * NL], "wq": wq, "wk": wk}
        for c in range(NC)
    ]
    r = run_bass_kernel_spmd(nc, in_maps, list(range(NC)))
    return np.concatenate([r.results[c]["out_local"] for c in range(NC)], axis=0)
